# revision 2
# baseline (speedup 1.0000x reference)
"""Causal self-attention with RoPE on 8 trn2 NeuronCores.

Sharding: core = (batch, head-half). Each of the 8 cores handles one batch
(b = core//2) and 6 of the 12 heads (hh = core%2). Each core computes a
partial output projection (its heads' contribution to y @ Wproj); the host
sums the two partials per batch.

Device kernel (identical SPMD program on every core):
  phase 1: v = x @ Wv (natural s-major layout, with an appended ones column
           per head for the softmax denominator), qT/kT = (x @ Wq/Wk)^T in
           d-major layout with RoPE applied via stream_shuffle (the head dim
           is pre-permuted host-side so rotate-half partners are adjacent
           even/odd lanes - scores are invariant to that permutation).
  phase 2: flash-style causal attention per head-pair: S^T blocks
           (k-partition x q-free) via row-packed K=64 matmuls, one exp per
           block (3D AP over both heads), diagonal mask multiply, and
           yT[65 x q] = [v|1]^T @ E accumulated in PSUM (row 64 = denom).
  phase 3: out_partial = yT_normalized^T @ Wp_rows.
"""
import contextlib

import numpy as np

import concourse.bacc as bacc
import concourse.mybir as mybir
import concourse.tile as tile
from concourse import bass_utils

F32 = mybir.dt.float32

B, S, C, H, D = 4, 2048, 768, 12, 64
HPC = H // 2          # heads per core = 6
HP = HPC // 2         # head pairs per core = 3
KC = C // 128         # contraction tiles over C = 6
NST = S // 128        # 128-row s tiles = 16
NSC = S // 512        # 512-wide s chunks = 4
ROPE_BASE = 10000.0

EVEN_ODD_MASK = [x for j in range(16) for x in (2 * j + 1, 2 * j)]


def build_program():
    nc = bacc.Bacc("TRN2", target_bir_lowering=False, debug=False)
    xT_d = nc.dram_tensor("xT", [C, S], F32, kind="ExternalInput").ap()
    wqk_d = nc.dram_tensor("wqk", [C, 768], F32, kind="ExternalInput").ap()
    wv_d = nc.dram_tensor("wv", [C, 384], F32, kind="ExternalInput").ap()
    wp_d = nc.dram_tensor("wp", [384, C], F32, kind="ExternalInput").ap()
    cos_d = nc.dram_tensor("cosT", [128, S], F32, kind="ExternalInput").ap()
    sin_d = nc.dram_tensor("sinA", [128, S], F32, kind="ExternalInput").ap()
    mask_d = nc.dram_tensor("mask01", [128, 128], F32, kind="ExternalInput").ap()
    out_d = nc.dram_tensor("out", [S, C], F32, kind="ExternalOutput").ap()

    with tile.TileContext(nc) as tc, contextlib.ExitStack() as top:
        res1 = top.enter_context(tc.tile_pool(name="res1", bufs=1))
        qkT = [res1.tile([128, S], F32, name=f"qkT{i}", tag=f"qkT{i}") for i in range(6)]
        vones = [res1.tile([128, HPC * 65], F32, name=f"vones{i}", tag=f"vones{i}") for i in range(NST)]
        mask01 = res1.tile([128, 128], F32, name="mask01", tag="mask01")
        nc.sync.dma_start(mask01[:], mask_d[:])

        # ---------------- phase 1: projections + RoPE ----------------
        with contextlib.ExitStack() as ph1:
            p1 = ph1.enter_context(tc.tile_pool(name="ph1", bufs=1))
            p1ps = ph1.enter_context(tc.tile_pool(name="ph1ps", bufs=1, space="PSUM"))
            xT = [p1.tile([128, S], F32, name=f"xT{i}", tag=f"xT{i}") for i in range(KC)]
            wqk = [p1.tile([128, 768], F32, name=f"wqk{i}", tag=f"wqk{i}") for i in range(KC)]
            wv = [p1.tile([128, 384], F32, name=f"wv{i}", tag=f"wv{i}") for i in range(KC)]
            cosT = p1.tile([128, S], F32, tag="cosT")
            sinA = p1.tile([128, S], F32, tag="sinA")
            for i in range(KC):
                nc.sync.dma_start(xT[i][:], xT_d[128 * i : 128 * (i + 1), :])
                nc.sync.dma_start(wqk[i][:], wqk_d[128 * i : 128 * (i + 1), :])
                nc.sync.dma_start(wv[i][:], wv_d[128 * i : 128 * (i + 1), :])
            nc.sync.dma_start(cosT[:], cos_d[:])
            nc.sync.dma_start(sinA[:], sin_d[:])

            # v projection -> vones tiles (s-major), ones col per head
            for st in range(NST):
                vps = p1ps.tile([128, 384], F32, tag="vps", bufs=2)
                for kc in range(KC):
                    nc.tensor.matmul(
                        vps[:],
                        xT[kc][:, 128 * st : 128 * (st + 1)],
                        wv[kc][:],
                        start=(kc == 0),
                        stop=(kc == KC - 1),
                    )
                vt = vones[st]
                nc.gpsimd.memset(vt[:], 1.0)
                dst = vt[:].rearrange("p (h w) -> p h w", w=65)[:, :, 0:64]
                src = vps[:].rearrange("p (h w) -> p h w", w=64)
                nc.vector.tensor_copy(dst, src)

            # q/k projection (d-major) + RoPE
            for m in range(6):
                for sc in range(NSC):
                    sl = slice(512 * sc, 512 * (sc + 1))
                    qkps = p1ps.tile([128, 512], F32, tag="qkps", bufs=3)
                    for kc in range(KC):
                        nc.tensor.matmul(
                            qkps[:],
                            wqk[kc][:, 128 * m : 128 * (m + 1)],
                            xT[kc][:, sl],
                            start=(kc == 0),
                            stop=(kc == KC - 1),
                        )
                    shuf = p1.tile([128, 512], F32, tag="shuf", bufs=3)
                    t1 = p1.tile([128, 512], F32, tag="t1", bufs=3)
                    nc.vector.stream_shuffle(shuf[:], qkps[:], EVEN_ODD_MASK)
                    nc.vector.tensor_mul(t1[:], qkps[:], cosT[:, sl])
                    nc.vector.tensor_mul(shuf[:], shuf[:], sinA[:, sl])
                    nc.vector.tensor_add(qkT[m][:, sl], t1[:], shuf[:])

        # ---------------- phase 2: causal attention ----------------
        res2 = top.enter_context(tc.tile_pool(name="res2", bufs=1))
        yTn = [res2.tile([128, S], F32, name=f"yTn{i}", tag=f"yTn{i}") for i in range(HP)]
        with contextlib.ExitStack() as ph2:
            p2 = ph2.enter_context(tc.tile_pool(name="ph2", bufs=1))
            p2ps = ph2.enter_context(tc.tile_pool(name="ph2ps", bufs=1, space="PSUM"))
            for hp in range(HP):
                qTt = qkT[hp]
                kTt = qkT[HP + hp]
                for c in range(NSC):
                    yps = [p2ps.tile([128, 512], F32, name="yps", tag="yT", bufs=4) for _ in range(2)]
                    for kb in range(4 * c + 4):
                        off = max(0, 128 * kb - 512 * c)
                        qsl = slice(512 * c + off, 512 * (c + 1))
                        ksl = slice(128 * kb, 128 * (kb + 1))
                        sT = p2ps.tile([128, 1024], F32, tag="sT", bufs=2)
                        nc.tensor.matmul(
                            sT[:, off:512], kTt[0:64, ksl], qTt[0:64, qsl],
                            start=True, stop=True, tile_position=(0, 0),
                        )
                        nc.tensor.matmul(
                            sT[:, 512 + off : 1024], kTt[64:128, ksl], qTt[64:128, qsl],
                            start=True, stop=True, tile_position=(64, 0),
                        )
                        eT = p2.tile([128, 1024], F32, tag="eT", bufs=3)
                        in3 = sT[:].rearrange("p (b w) -> p b w", b=2)[:, :, off:512]
                        out3 = eT[:].rearrange("p (b w) -> p b w", b=2)[:, :, off:512]
                        nc.scalar.activation(
                            out3, in3, mybir.ActivationFunctionType.Exp, scale=D**-0.5
                        )
                        if kb >= 4 * c:  # diagonal block: causal mask multiply
                            for h in range(2):
                                dsl = slice(512 * h + off, 512 * h + off + 128)
                                nc.gpsimd.tensor_mul(eT[:, dsl], eT[:, dsl], mask01[:])
                        for h in range(2):
                            nc.tensor.matmul(
                                yps[h][0:65, off:512],
                                vones[kb][:, 65 * (2 * hp + h) : 65 * (2 * hp + h) + 65],
                                eT[:, 512 * h + off : 512 * (h + 1)],
                                start=(kb == 0),
                                stop=(kb == 4 * c + 3),
                            )
                    for h in range(2):
                        recip = p2.tile([1, 512], F32, tag="recip", bufs=2)
                        nc.vector.reciprocal(recip[:], yps[h][64:65, 0:512])
                        bc = p2.tile([64, 512], F32, tag="bc", bufs=2)
                        nc.gpsimd.partition_broadcast(bc[:], recip[:], channels=64)
                        nc.vector.tensor_mul(
                            yTn[hp][64 * h : 64 * (h + 1), 512 * c : 512 * (c + 1)],
                            yps[h][0:64, 0:512],
                            bc[:],
                        )

        # ---------------- phase 3: output projection ----------------
        with contextlib.ExitStack() as ph3:
            p3 = ph3.enter_context(tc.tile_pool(name="ph3", bufs=1))
            p3ps = ph3.enter_context(tc.tile_pool(name="ph3ps", bufs=1, space="PSUM"))
            wp = [p3.tile([128, 768], F32, name=f"wp{i}", tag=f"wp{i}") for i in range(HP)]
            for i in range(HP):
                nc.sync.dma_start(wp[i][:], wp_d[128 * i : 128 * (i + 1), :])
            for st in range(NST):
                osb = p3.tile([128, 768], F32, tag="osb", bufs=3)
                for half in range(2):
                    ops_ = p3ps.tile([128, 384], F32, tag="ops", bufs=4)
                    for t in range(HP):
                        nc.tensor.matmul(
                            ops_[:],
                            yTn[t][:, 128 * st : 128 * (st + 1)],
                            wp[t][:, 384 * half : 384 * (half + 1)],
                            start=(t == 0),
                            stop=(t == HP - 1),
                        )
                    nc.vector.tensor_copy(osb[:, 384 * half : 384 * (half + 1)], ops_[:])
                nc.sync.dma_start(out_d[128 * st : 128 * (st + 1), :], osb[:])

    nc.compile()
    return nc


def _rope_tables():
    """cosT/sinA in the even/odd-interleaved d order, tiled to 128 partitions."""
    j = np.arange(32, dtype=np.float64)
    theta = ROPE_BASE ** (-2.0 * j / D)
    pos = np.arange(S, dtype=np.float64)
    freqs = np.outer(theta, pos)  # (32, S)
    cos = np.cos(freqs)
    sin = np.sin(freqs)
    cosT = np.empty((64, S), np.float32)
    sinA = np.empty((64, S), np.float32)
    cosT[0::2] = cos
    cosT[1::2] = cos
    sinA[0::2] = -sin
    sinA[1::2] = sin
    return np.tile(cosT, (2, 1)).copy(), np.tile(sinA, (2, 1)).copy()


def _head_perm():
    """Even/odd interleave of RoPE partner dims, per head (384 cols)."""
    perm = np.empty(384, np.int64)
    for h in range(HPC):
        for j in range(32):
            perm[64 * h + 2 * j] = 64 * h + j
            perm[64 * h + 2 * j + 1] = 64 * h + j + 32
    return perm


def make_in_maps(x, Wqkv, Wproj):
    x = np.asarray(x, np.float32)
    Wqkv = np.asarray(Wqkv, np.float32)
    Wproj = np.asarray(Wproj, np.float32)
    wq, wk, wv = Wqkv[:, 0:C], Wqkv[:, C : 2 * C], Wqkv[:, 2 * C : 3 * C]
    cosT, sinA = _rope_tables()
    perm = _head_perm()
    mask01 = (np.arange(128)[None, :] >= np.arange(128)[:, None]).astype(np.float32)
    in_maps = []
    for core in range(8):
        b, hh = core // 2, core % 2
        cols = slice(384 * hh, 384 * (hh + 1))
        wq_c = wq[:, cols][:, perm]
        wk_c = wk[:, cols][:, perm]
        in_maps.append(
            {
                "xT": np.ascontiguousarray(x[b].T),
                "wqk": np.ascontiguousarray(np.concatenate([wq_c, wk_c], axis=1)),
                "wv": np.ascontiguousarray(wv[:, cols]),
                "wp": np.ascontiguousarray(Wproj[384 * hh : 384 * (hh + 1), :]),
                "cosT": cosT,
                "sinA": sinA,
                "mask01": mask01,
            }
        )
    return in_maps


_NC_CACHE = None


def _get_program():
    global _NC_CACHE
    if _NC_CACHE is None:
        _NC_CACHE = build_program()
    return _NC_CACHE


def kernel(x, Wqkv, Wproj):
    nc = _get_program()
    in_maps = make_in_maps(x, Wqkv, Wproj)
    res = bass_utils.run_bass_kernel_spmd(nc, in_maps, core_ids=list(range(8)))
    out = np.empty((B, S, C), np.float32)
    for b in range(B):
        out[b] = res.results[2 * b]["out"] + res.results[2 * b + 1]["out"]
    return out
